# revision 10
# baseline (speedup 1.0000x reference)
"""Self-attention kernel for TRN2: out = softmax(X Wq (X Wk)^T / sqrt(D)) @ X.

Strategy (8-way sequence parallelism over query rows):
  scores = X M X^T with M = Wq Wk^T / sqrt(D) folded ON THE HOST (weight
fusion: M depends only on weights). Each core i handles query rows
[i*B, (i+1)*B):
  phase 0 (ONE streamed GEMM):  A^T = M^T X_i^T   [D, B]
    hh pass in f32r: Mh x Xh, both host-rounded to 11-bit mantissas (the
    PE's f32r read path keeps an e10m11 operand, so 11-bit-rounded values
    pass through exactly; products are exact in the fp32 PSUM).
    cross pass in ONE fp8 DoubleRow sweep using mixed pairs
      w = (Ml*2^9, Mh*2^2), m = (Xh*2^4, Xl*2^11):
    each DR cell computes Ml*Xh + Mh*Xl summed = both cross terms at
    once, pre-scaled by 2^13 (so all four fp8 operands sit in e4m3's
    normal range); descaled on ScalarE and added to the hh PSUM on DVE.
    outputs: A^T as f32r hi tiles (aith, device-rounded by the f32r cast)
    + e4m3 cross operands h8a = hi(A)*2^-9, l8a = lo(A)*2^4 (DR layout).
  flash:   stream key blocks j; logits S^T_j = X_j A in key-major layout as
           f32r hh pass + TWO fp8 DoubleRow cross passes:
             term1 = lo(X)*2^9 (stationary) x hi(A)*2^-9 (moving)
             term2 = hi(X)*2^-4 x lo(A)*2^4
           running column-max via GpSimd partition_all_reduce(max) (no PE
           transposes); E = exp(S - max) in bf16; out matmul E^T-slices @ X
           (bf16); softmax denominator via GpSimd tile-sums + partition
           all-reduce (no PE ones-matmuls); fused rescale-accumulate
           (acc = acc*corr + psum) on DVE; final divide.
           corr/max bookkeeping lives in [1,B] row space; the per-query-tile
           rescale scalar comes from one small DRAM bounce + one PE
           transpose per super-block, issued AFTER the out burst so the PE
           never stalls on the GpSimd reduce chain.

Numerics: logits need ~17-bit abs precision (std ~1024, near-tie rows
amplify errors through softmax). f32r hh + e4m3 cross terms leave ~8e-3
rms logit noise (measured in numpy emulation of the exact device quantizer
chain) -- under the ~0.04 budget (gate 2e-2 on max-abs/max-scale). The P@X
matmul only needs ~1e-3 relative, so bf16 with denominator cancellation is
safe there.

Software pipeline: PE queue per super-block is [S(s) | out(s-1) | corr
transpose(s)]; DVE queue is [drain-maxes(s) | rescale-STTs(s-1) | row
stats(s) | exp-subs(s)] so nothing head-of-line blocks on the GpSimd
reduce; GpSimd runs [max-reduce(s) | den-sums(s-1) | den-reduce(s-1)].
"""
import numpy as np
from contextlib import ExitStack

import concourse.bass as bass
import concourse.bacc as bacc
import concourse.tile as tile
from concourse import mybir
from concourse.bass_utils import run_bass_kernel_spmd
from concourse.masks import make_identity
from bass_rust import ReduceOp

P = 128
SEQ = 8192
DIM = 1024
NCORES = 8
SBN = 4      # key n-tiles (of 128) per flash super-block

F32 = mybir.dt.float32
F32R = mybir.dt.float32r
BF16 = mybir.dt.bfloat16
F8 = mybir.dt.float8e4
EXP = mybir.ActivationFunctionType.Exp
ALU = mybir.AluOpType
AXX = mybir.AxisListType.X
DR = mybir.MatmulPerfMode.DoubleRow

# flash fp8 cross-term scales (product of each pair = 1.0)
S_LX = 512.0        # lo(X) * 2^9   (stationary, term1)
S_HA = 1.0 / 512.0  # hi(A) * 2^-9  (moving, term1)
S_HX = 1.0 / 16.0   # hi(X) * 2^-4  (stationary, term2)
S_LA = 16.0         # lo(A) * 2^4   (moving, term2)

# phase-0 fp8 mixed-pair scales: products net to 2^13, descaled on ScalarE
S_ML = 512.0        # Ml * 2^9   (w pair elem 0)
S_XH8 = 16.0        # Xh * 2^4   (m pair elem 0)
S_MH8 = 4.0         # Mh * 2^2   (w pair elem 1)
S_XL8 = 2048.0      # Xl * 2^11  (m pair elem 1)
CROSS_DESCALE = 2.0 ** -13


def _chunks(total, step=512):
    return [(lo, min(lo + step, total)) for lo in range(0, total, step)]


def build_core_kernel(S, D, B, sbn=SBN):
    """One core's kernel: query rows block of size B, full S keys."""
    KT = D // P      # contraction tiles over D
    NT = S // P      # key tiles
    MT = B // P      # query tiles (per core)
    NSB = NT // sbn  # super-blocks
    NPAIR = KT // 2  # DoubleRow contraction pairs
    assert NT % sbn == 0 and B % P == 0 and D % P == 0 and MT <= P and KT % 2 == 0

    nc = bacc.Bacc("TRN2", target_bir_lowering=False, debug=False)
    xtj = nc.dram_tensor("xtj", [NT, P, D], F32, kind="ExternalInput")
    xl8 = nc.dram_tensor("xl8", [NT, P, D], F8, kind="ExternalInput")
    xh8 = nc.dram_tensor("xh8", [NT, P, D], F8, kind="ExternalInput")
    xa = nc.dram_tensor("xa", [S, D], BF16, kind="ExternalInput")
    mh = nc.dram_tensor("mh", [KT, P, KT, P], F32, kind="ExternalInput")
    m8 = nc.dram_tensor("m8", [KT, P, KT, 2, P], F8, kind="ExternalInput")
    xih = nc.dram_tensor("xih", [KT, P, B], F32, kind="ExternalInput")
    x8p = nc.dram_tensor("x8p", [KT, P, 2, B], F8, kind="ExternalInput")
    out = nc.dram_tensor("out", [B, D], F32, kind="ExternalOutput")
    dscr = nc.dram_tensor("dscr", [1, B], F32, kind="Internal")
    dcorr = nc.dram_tensor("dcorr", [1, B], F32, kind="Internal")

    def pair_st(t, u):
        # stationary fp8 pair view [P, 2, P] of a [P, D] tile, pair u
        return t[:, u * 2 * P:(u + 1) * 2 * P].rearrange("p (i m) -> p i m", i=2)

    with tile.TileContext(nc) as tc, ExitStack() as ctx:
        pers = ctx.enter_context(tc.tile_pool(name="pers", bufs=1))
        aith = [pers.tile([P, B], F32R, name=f"aith{k}") for k in range(KT)]
        h8a = pers.tile([P, KT, B], F8, name="h8a")
        l8a = pers.tile([P, KT, B], F8, name="l8a")
        gm = pers.tile([P, B], F32, name="gm")
        mxbc = pers.tile([P, B], F32, name="mxbc")
        ident = pers.tile([P, P], F32, name="ident")
        den = pers.tile([1, B], F32, name="den")
        omx_row = pers.tile([1, B], F32, name="omx_row")
        ctmp = pers.tile([P, P], F32, name="ctmp")
        make_identity(nc, ident[:])
        nc.gpsimd.memset(gm[:], -1e30)
        nc.gpsimd.memset(den[:], 0.0)
        nc.gpsimd.memset(ctmp[:], 0.0)

        # ---- phase 0: A^T = M^T X_i^T (hh f32r + one fp8 DR cross sweep) ----
        with ExitStack() as p0:
            wp = p0.enter_context(tc.tile_pool(name="wp", bufs=2))
            xsp = p0.enter_context(tc.tile_pool(name="xsp", bufs=3))
            xip = p0.enter_context(tc.tile_pool(name="xip", bufs=1))
            atp = p0.enter_context(tc.tile_pool(name="atp", bufs=2))
            ps0 = p0.enter_context(tc.tile_pool(name="ps0", bufs=6, space="PSUM"))

            # d=0 weight DMA + cast first on their queues: the first matmul
            # must not wait behind the X streams
            wf0 = wp.tile([P, KT, P], F32, name="wf0", tag="wf")
            nc.sync.dma_start(wf0[:], mh.ap()[0])
            hw0 = wp.tile([P, KT, P], F32R, name="hw0", tag="hw")
            nc.scalar.copy(hw0[:], wf0[:])
            m8t0 = wp.tile([P, KT, 2, P], F8, name="m8t0", tag="m8t")
            nc.sync.dma_start(m8t0[:], m8.ap()[0])

            # X_i^T moving operands, streamed HALF-FIRST so the d-loop's
            # half-0 matmuls start after only B/2 columns of X have landed
            hxi = xip.tile([P, KT, B], F32R, name="hxi")
            x8t = xip.tile([P, KT, 2, B], F8, name="x8t")
            for (lo, hi) in _chunks(B):
                for g in range(KT):
                    stg = xsp.tile([P, 512], F32, name=f"xstg{g}_{lo}", tag="xstg")
                    nc.sync.dma_start(stg[:], xih.ap()[g, :, lo:hi])
                    nc.scalar.copy(hxi[:, g, lo:hi], stg[:])
                    nc.sync.dma_start(x8t[:, g, :, lo:hi], x8p.ap()[g, :, :, lo:hi])

            for d in range(KT):
                if d == 0:
                    hw, m8t = hw0, m8t0
                else:
                    wf = wp.tile([P, KT, P], F32, name=f"wf{d}", tag="wf")
                    nc.sync.dma_start(wf[:], mh.ap()[d])
                    hw = wp.tile([P, KT, P], F32R, name=f"hw{d}", tag="hw")
                    nc.scalar.copy(hw[:], wf[:])
                    m8t = wp.tile([P, KT, 2, P], F8, name=f"m8t{d}", tag="m8t")
                    nc.sync.dma_start(m8t[:], m8.ap()[d])
                for (lo, hi) in _chunks(B):
                    ph = ps0.tile([P, 512], F32, name=f"ph{d}_{lo}", tag="pm")
                    pc = ps0.tile([P, 512], F32, name=f"pc{d}_{lo}", tag="pm")
                    for r in range(KT):
                        nc.tensor.matmul(ph[:], hw[:, r, :], hxi[:, r, lo:hi],
                                         start=(r == 0), stop=(r == KT - 1))
                    for r in range(KT):
                        nc.tensor.matmul(pc[:], m8t[:, r, :, :],
                                         x8t[:, r, :, lo:hi],
                                         start=(r == 0), stop=(r == KT - 1),
                                         perf_mode=DR)
                    at = atp.tile([P, 512], F32, name=f"at{d}_{lo}", tag="at")
                    # two PSUM reads aren't allowed in one instruction:
                    # ScalarE drains the cross PSUM (with the descale),
                    # DVE adds the hh PSUM
                    nc.scalar.mul(at[:], pc[:], CROSS_DESCALE)
                    nc.vector.tensor_add(at[:], at[:], ph[:])
                    nc.vector.tensor_copy(aith[d][:, lo:hi], at[:])
                    nc.scalar.mul(h8a[:, d, lo:hi],
                                  aith[d][:, lo:hi].bitcast(F32), S_HA)
                    nc.vector.tensor_sub(at[:], at[:],
                                         aith[d][:, lo:hi].bitcast(F32))
                    nc.vector.tensor_scalar_mul(l8a[:, d, lo:hi], at[:], S_LA)

        # ---- flash over key super-blocks ----
        accp = ctx.enter_context(tc.tile_pool(name="accp", bufs=1))
        acc = [accp.tile([P, D], F32, name=f"acc{t}") for t in range(MT)]
        for t in range(MT):
            nc.gpsimd.memset(acc[t][:], 0.0)
        sp = ctx.enter_context(tc.tile_pool(name="sp", bufs=2 * sbn + 2))
        erp = ctx.enter_context(tc.tile_pool(name="erp", bufs=2 * sbn))
        xap = ctx.enter_context(tc.tile_pool(name="xap", bufs=3))
        xarp = ctx.enter_context(tc.tile_pool(name="xarp", bufs=2 * sbn))
        xthp = ctx.enter_context(tc.tile_pool(name="xthp", bufs=3))
        stat = ctx.enter_context(tc.tile_pool(name="stat", bufs=2))
        ps_s = ctx.enter_context(tc.tile_pool(name="ps_s", bufs=4, space="PSUM"))
        ps_o = ctx.enter_context(tc.tile_pool(name="ps_o", bufs=2, space="PSUM"))
        ps_t = ps_s

        def prep_block(s):
            xsplit = []
            for j in range(s * sbn, (s + 1) * sbn):
                xt_t = xap.tile([P, D], F32, name=f"xt{j}", tag="stg")
                nc.sync.dma_start(xt_t[:], xtj.ap()[j])
                xth = xthp.tile([P, D], F32R, name=f"xth{j}", tag="xth")
                nc.scalar.copy(xth[:], xt_t[:])
                l8x = xthp.tile([P, D], F8, name=f"l8x{j}", tag="l8x")
                nc.sync.dma_start(l8x[:], xl8.ap()[j])
                h8x = xthp.tile([P, D], F8, name=f"h8x{j}", tag="h8x")
                nc.sync.dma_start(h8x[:], xh8.ap()[j])
                xsplit.append((xth, l8x, h8x))
            return xsplit

        def s_burst(s, xsplit):
            ssb = []
            for idx, j in enumerate(range(s * sbn, (s + 1) * sbn)):
                xth, l8x, h8x = xsplit[idx]
                s_t = sp.tile([P, B], F32, name=f"s{j}", tag="s")
                pss = [ps_s.tile([P, 512], F32, name=f"pss{j}_{c}", tag="pss")
                       for c in range(2)]
                # f32r hh pass, both chunks back-to-back (same PE mode)
                for c, (lo, hi) in enumerate(_chunks(B)):
                    for k in range(KT):
                        nc.tensor.matmul(pss[c][:], xth[:, k * P:(k + 1) * P],
                                         aith[k][:, lo:hi], start=(k == 0), stop=(k == KT - 1))
                # fp8 DoubleRow cross passes accumulate onto the closed f32r
                # group via has_written (start=False); chunk 0 drains (copy +
                # running max) while chunk 1's fp8 matmuls stream.
                for c, (lo, hi) in enumerate(_chunks(B)):
                    for u in range(NPAIR):
                        nc.tensor.matmul(pss[c][:], pair_st(l8x, u),
                                         h8a[:, 2 * u:2 * u + 2, lo:hi],
                                         start=False, stop=True, perf_mode=DR,
                                         skip_group_check=True)
                    for u in range(NPAIR):
                        nc.tensor.matmul(pss[c][:], pair_st(h8x, u),
                                         l8a[:, 2 * u:2 * u + 2, lo:hi],
                                         start=False, stop=True, perf_mode=DR,
                                         skip_group_check=True)
                    nc.scalar.copy(s_t[:, lo:hi], pss[c][:])
                    nc.vector.tensor_max(gm[:, lo:hi], gm[:, lo:hi], pss[c][:])
                ssb.append(s_t)
            return ssb

        def stats_row(s):
            # [1,B]-space stats AFTER the out burst is queued on DVE, so the
            # wait on the GpSimd reduce never head-of-line blocks the queue
            corr_row = stat.tile([1, B], F32, name=f"crow{s}", tag="crow")
            if s == 0:
                nc.vector.memset(corr_row[:], 0.0)
            else:
                nc.vector.tensor_sub(corr_row[:], omx_row[:], mxbc[0:1, :])
                nc.scalar.activation(corr_row[:], corr_row[:], EXP)
                # bounce corr_row -> query-major staging (scalar DMA queue:
                # FIFO write-then-read through DRAM, off the bulk sync queue)
                nc.scalar.dma_start(dcorr.ap()[:, :], corr_row[:])
                nc.scalar.dma_start(ctmp[:MT, :],
                                    dcorr.ap()[0, :].rearrange("(b c) -> b c", b=MT))
            nc.vector.tensor_copy(omx_row[:], mxbc[0:1, :])
            return corr_row

        def stats_qm(s):
            corr_qm = stat.tile([P, MT], F32, name=f"cqm{s}", tag="cqm")
            if s == 0:
                nc.vector.memset(corr_qm[:], 0.0)
            else:
                ptc = ps_t.tile([P, P], F32, name=f"ptc{s}", tag="pss")
                nc.tensor.transpose(ptc[:], ctmp[:], ident[:])
                nc.vector.tensor_copy(corr_qm[:], ptc[:, :MT])
            return corr_qm

        def exp_block(s, ssb):
            # E = exp(S - max), exp writes bf16 er (out dtype converts).
            # Chunk 0 of every tile first: the out burst consumes er columns
            # in t order, so its early stationaries come from chunk 0.
            ers = [erp.tile([P, B], BF16, name=f"er{s}_{idx}", tag="er")
                   for idx in range(len(ssb))]
            for (lo, hi) in _chunks(B):
                for idx, s_t in enumerate(ssb):
                    nc.vector.tensor_sub(s_t[:, lo:hi], s_t[:, lo:hi], mxbc[:, lo:hi])
                    nc.scalar.activation(ers[idx][:, lo:hi], s_t[:, lo:hi], EXP)
            return ers

        def xar_block(s):
            xar = []
            for j in range(s * sbn, (s + 1) * sbn):
                xa_t = xarp.tile([P, D], BF16, name=f"xa{j}", tag="xar")
                nc.sync.dma_start(xa_t[:], xa.ap()[j * P:(j + 1) * P, :])
                xar.append(xa_t)
            return xar

        def den_update(s, ers, corr_row, dsum):
            # softmax denominator: GpSimd tile-sums + partition all-reduce
            # (replaces PE ones-matmuls). dsum is a dead s_t score tile from
            # this super-block, reused as scratch.
            nc.gpsimd.tensor_add(dsum[:], ers[0][:], ers[1][:])
            for idx in range(2, sbn):
                nc.gpsimd.tensor_add(dsum[:], dsum[:], ers[idx][:])
            nc.gpsimd.partition_all_reduce(dsum[:], dsum[:], P, ReduceOp.add)
            nc.vector.tensor_mul(den[:], den[:], corr_row[:])
            nc.vector.tensor_add(den[:], den[:], dsum[0:1, :])

        def out_block(s, ers, xar, corr_row, corr_qm, dsum, final=False):
            # out accumulation: acc = acc*corr + E^T @ X (bf16 burst).
            # On the final block the denominator runs FIRST so its
            # row->query-major DRAM bounce finishes during the burst and the
            # per-tile divides+stores pipeline with the matmuls.
            rcd = None
            if final:
                den_update(s, ers, corr_row, dsum)
                nc.sync.dma_start(dscr.ap()[:, :], den[:])
                dtmp = stat.tile([P, P], F32, name="dtmp", tag="dtmp", bufs=1)
                nc.gpsimd.memset(dtmp[:], 0.0)
                nc.sync.dma_start(dtmp[:MT, :], dscr.ap()[0, :].rearrange("(b c) -> b c", b=MT))
            for t in range(MT):
                po = ps_o.tile([P, D], F32, name=f"po{s}_{t}", tag="po")
                # idx outer so the column chunks reuse one stationary
                # operand back-to-back (LDWEIGHTS locality)
                for idx in range(sbn):
                    er = ers[idx][:]
                    for (lo, hi) in _chunks(D):
                        nc.tensor.matmul(po[:, lo:hi], er[:, t * P:(t + 1) * P],
                                         xar[idx][:, lo:hi], start=(idx == 0), stop=(idx == sbn - 1))
                nc.vector.scalar_tensor_tensor(acc[t][:], acc[t][:],
                                               corr_qm[:, t:t + 1], po[:],
                                               op0=ALU.mult, op1=ALU.add)
                if final and t == 2:
                    ptd = ps_t.tile([P, P], F32, name="ptd", tag="pss")
                    nc.tensor.transpose(ptd[:], dtmp[:], ident[:])
                    rcd = stat.tile([P, MT], F32, name="rcd", tag="rcd", bufs=1)
                    nc.vector.reciprocal(rcd[:], ptd[:, :MT])
                if final and t >= 2:
                    for tt in ([0, 1, 2] if t == 2 else [t]):
                        nc.vector.tensor_scalar_mul(acc[tt][:], acc[tt][:], rcd[:, tt:tt + 1])
                        eng = nc.sync if tt % 2 == 0 else nc.scalar
                        eng.dma_start(out.ap()[tt * P:(tt + 1) * P, :], acc[tt][:])
            if not final:
                den_update(s, ers, corr_row, dsum)

        prev = None     # out_block args for block s-1
        xsplit = prep_block(0)
        for s in range(NSB):
            ssb = s_burst(s, xsplit)
            if s + 1 < NSB:
                xsplit = prep_block(s + 1)
            # running per-query max: GpSimd all-reduce over partitions,
            # issued first so it runs under the out burst
            nc.gpsimd.partition_all_reduce(mxbc[:], gm[:], P, ReduceOp.max)
            if prev is not None:
                out_block(s - 1, *prev)
            corr_row = stats_row(s)
            corr_qm = stats_qm(s)
            ers = exp_block(s, ssb)
            xar = xar_block(s)
            prev = (ers, xar, corr_row, corr_qm, ssb[0])
        out_block(NSB - 1, *prev, final=True)

    nc.compile()
    return nc


def _split_f32r(x):
    """Host replica of the f32r hi/lo split: hi = x rounded (half-up) to an
    11-bit mantissa — so the PE's e10m11 operand truncation and the
    on-device f32r casts both read it back exactly — and lo = x - hi,
    exact in fp32."""
    x = np.ascontiguousarray(x, np.float32)
    hi = ((x.view(np.uint32) + np.uint32(0x800)) & np.uint32(0xFFFFF000)).view(np.float32)
    return hi, (x - hi).astype(np.float32)


def _to_f8(x):
    import ml_dtypes
    return np.clip(x, -240.0, 240.0).astype(ml_dtypes.float8_e4m3)


def prep_inputs(X, Wq, Wk, S, D, n_cores):
    import ml_dtypes
    B = S // n_cores
    NT = S // P
    KT = D // P
    X = np.ascontiguousarray(X, np.float32)
    scale = 1.0 / np.sqrt(D)

    # flash operands (keys side): X transposed tiles, 11-bit hi + fp8 splits
    xtj = np.ascontiguousarray(
        X.reshape(NT, P, KT, P).transpose(0, 3, 2, 1).reshape(NT, P, D))
    xtj_hi, xtj_lo = _split_f32r(xtj)
    xl8 = _to_f8(xtj_lo * np.float32(S_LX))
    xh8 = _to_f8(xtj_hi * np.float32(S_HX))
    xa = X.astype(ml_dtypes.bfloat16)

    # host weight fusion: M = (Wq/sqrt(D)) @ Wk^T in fp64, then 11-bit hi
    M = np.asarray(
        (np.asarray(Wq, np.float64) * scale) @ np.asarray(Wk, np.float64).T,
        np.float32)
    mhi, mlo = _split_f32r(M)
    # device layouts: mh_dev[d, p, g, c] = Mh[g*P+p, d*P+c] (contiguous per d)
    def to_dpgc(W):
        return np.ascontiguousarray(
            W.reshape(KT, P, KT, P).transpose(2, 1, 0, 3))
    mh_dev = to_dpgc(mhi)
    m8_dev = np.ascontiguousarray(np.stack(
        [to_dpgc(_to_f8(mlo * np.float32(S_ML))),
         to_dpgc(_to_f8(mhi * np.float32(S_MH8)))], axis=3))

    # phase-0 moving operands: X^T core slices, 11-bit hi + fp8 pair
    XT = np.ascontiguousarray(X.T)
    xih_full, xil_full = _split_f32r(XT)
    xh8p_full = _to_f8(xih_full * np.float32(S_XH8))
    xl8p_full = _to_f8(xil_full * np.float32(S_XL8))

    in_maps = []
    for i in range(n_cores):
        sl = slice(i * B, (i + 1) * B)
        in_maps.append({
            "xtj": xtj_hi, "xl8": xl8, "xh8": xh8, "xa": xa,
            "mh": mh_dev, "m8": m8_dev,
            "xih": np.ascontiguousarray(xih_full[:, sl]).reshape(KT, P, B),
            "x8p": np.ascontiguousarray(np.stack(
                [xh8p_full[:, sl], xl8p_full[:, sl]],
                axis=1).reshape(KT, P, 2, B)),
        })
    return in_maps


_CACHE = {}


def _get_kernel(S, D, B, sbn):
    key = (S, D, B, sbn)
    if key not in _CACHE:
        _CACHE[key] = build_core_kernel(S, D, B, sbn=sbn)
    return _CACHE[key]


def kernel(inputs, weight_query, weight_key):
    S, D = inputs.shape
    assert (S, D) == (SEQ, DIM)
    B = S // NCORES
    nc = _get_kernel(S, D, B, SBN)
    in_maps = prep_inputs(inputs, weight_query, weight_key, S, D, NCORES)
    res = run_bass_kernel_spmd(nc, in_maps, core_ids=list(range(NCORES)))
    return np.concatenate([res.results[i]["out"] for i in range(NCORES)], axis=0)


if __name__ == "__main__":
    rng = np.random.default_rng(0)
    X = rng.standard_normal((SEQ, DIM), dtype=np.float32)
    Wq = rng.standard_normal((DIM, DIM), dtype=np.float32)
    Wk = rng.standard_normal((DIM, DIM), dtype=np.float32)
    out = kernel(X, Wq, Wk)
    print(out.shape, out.dtype)


# revision 11
# speedup vs baseline: 1.4760x; 1.4760x over previous
"""Self-attention kernel for TRN2: out = softmax(X Wq (X Wk)^T / sqrt(D)) @ X.

Strategy (8-way sequence parallelism over query rows):
  scores = X M X^T with M = Wq Wk^T / sqrt(D) folded ON THE HOST (weight
fusion: M depends only on weights, so the two projection GEMMs collapse
into one). Each core i handles query rows [i*B, (i+1)*B):
  phase 0 (ONE streamed GEMM):  A^T = M^T X_i^T   [D, B]
    hh pass in f32r: Mh x Xh, both host-rounded to 11-bit mantissas (the
    PE's f32r read path keeps an e10m11 operand, so 11-bit-rounded values
    pass through exactly; products are exact in the fp32 PSUM).
    cross pass in ONE fp8 DoubleRow sweep using mixed pairs
      w = (Ml*2^9, Mh*2^2), m = (Xh*2^4, Xl*2^11):
    each DR cell computes Ml*Xh + Mh*Xl summed = both cross terms at
    once, pre-scaled by 2^13 (so all four fp8 operands sit in e4m3's
    normal range); descaled on ScalarE and added to the hh PSUM on DVE.
    X_i^T streams half-B-first so the d-loop starts after 3MB, not 6MB.
    outputs: A^T as f32r hi tiles (aith, device-rounded by the f32r cast)
    + e4m3 cross operands h8a = hi(A)*2^-9, l8a = lo(A)*2^4 in DoubleRow
    pair layout.
  flash:   stream key blocks j; logits S^T_j = X_j A in key-major layout as
           f32r hh pass + TWO fp8 DoubleRow cross passes (each contracts 256
           per instruction = half the instructions of an f32r pass):
             term1 = lo(X)*2^9 (stationary) x hi(A)*2^-9 (moving)
             term2 = hi(X)*2^-4 x lo(A)*2^4
           running column-max via PE transpose + reduce; E = exp(S - max)
           in bf16; out matmul E^T-slices @ X (bf16); the softmax
           denominator via ones-stationary matmuls into a [1,B] row, with
           the same rescale chain (crow = corr transposed); fused
           rescale-accumulate (acc = acc*corr + psum) on DVE; final divide.

Numerics: logits need ~17-bit abs precision (std ~1024, near-tie rows
amplify errors through softmax). f32r hh + e4m3 cross terms leave ~8e-3
rms logit noise (measured in numpy emulation of the exact device quantizer
chain) -- under the ~0.04 budget (gate 2e-2 on max-abs/max-scale). The P@X
matmul only needs ~1e-3 relative, so bf16 with denominator cancellation is
safe there. Phase-0's fp8 crosses land on A before the ~sqrt(D)
amplification into the logits, adding ~5e-3; the 2^13 pre-scale keeps all
four fp8 operand distributions inside e4m3's normal range.

Software pipeline: PE queue per super-block is [S(s) | out(s-1) |
transposes(s)] with the max-broadcast/exp/xar chains on DVE/ScalarE/GpSimd
hidden under the bursts (stats stay PE-local: a GpSimd partition-reduce on
the critical path measures 6-8us and stalls the PE, the transposes don't).
"""
import numpy as np
from contextlib import ExitStack

import concourse.bass as bass
import concourse.bacc as bacc
import concourse.tile as tile
from concourse import mybir
from concourse.bass_utils import run_bass_kernel_spmd
from concourse.masks import make_identity

P = 128
SEQ = 8192
DIM = 1024
NCORES = 8
SBN = 4      # key n-tiles (of 128) per flash super-block

F32 = mybir.dt.float32
F32R = mybir.dt.float32r
BF16 = mybir.dt.bfloat16
F8 = mybir.dt.float8e4
EXP = mybir.ActivationFunctionType.Exp
ALU = mybir.AluOpType
AXX = mybir.AxisListType.X
DR = mybir.MatmulPerfMode.DoubleRow

# flash fp8 cross-term scales (product of each pair = 1.0)
S_LX = 512.0        # lo(X) * 2^9   (stationary, term1)
S_HA = 1.0 / 512.0  # hi(A) * 2^-9  (moving, term1)
S_HX = 1.0 / 16.0   # hi(X) * 2^-4  (stationary, term2)
S_LA = 16.0         # lo(A) * 2^4   (moving, term2)

# phase-0 fp8 mixed-pair scales: products net to 2^13, descaled on ScalarE
S_ML = 512.0        # Ml * 2^9   (w pair elem 0)
S_XH8 = 16.0        # Xh * 2^4   (m pair elem 0)
S_MH8 = 4.0         # Mh * 2^2   (w pair elem 1)
S_XL8 = 2048.0      # Xl * 2^11  (m pair elem 1)
CROSS_DESCALE = 2.0 ** -13


def _chunks(total, step=512):
    return [(lo, min(lo + step, total)) for lo in range(0, total, step)]


def build_core_kernel(S, D, B, sbn=SBN):
    """One core's kernel: query rows block of size B, full S keys."""
    KT = D // P      # contraction tiles over D
    NT = S // P      # key tiles
    MT = B // P      # query tiles (per core)
    NSB = NT // sbn  # super-blocks
    NPAIR = KT // 2  # DoubleRow contraction pairs
    assert NT % sbn == 0 and B % P == 0 and D % P == 0 and MT <= P and KT % 2 == 0

    nc = bacc.Bacc("TRN2", target_bir_lowering=False, debug=False)
    xtj = nc.dram_tensor("xtj", [NT, P, D], F32, kind="ExternalInput")
    xl8 = nc.dram_tensor("xl8", [NT, P, D], F8, kind="ExternalInput")
    xh8 = nc.dram_tensor("xh8", [NT, P, D], F8, kind="ExternalInput")
    xa = nc.dram_tensor("xa", [S, D], BF16, kind="ExternalInput")
    mh = nc.dram_tensor("mh", [KT, P, KT, P], F32, kind="ExternalInput")
    m8 = nc.dram_tensor("m8", [KT, P, KT, 2, P], F8, kind="ExternalInput")
    xih = nc.dram_tensor("xih", [KT, P, B], F32, kind="ExternalInput")
    x8p = nc.dram_tensor("x8p", [KT, P, 2, B], F8, kind="ExternalInput")
    out = nc.dram_tensor("out", [B, D], F32, kind="ExternalOutput")
    dscr = nc.dram_tensor("dscr", [1, B], F32, kind="Internal")

    def pair_st(t, u):
        # stationary fp8 pair view [P, 2, P] of a [P, D] tile, pair u
        return t[:, u * 2 * P:(u + 1) * 2 * P].rearrange("p (i m) -> p i m", i=2)

    with tile.TileContext(nc) as tc, ExitStack() as ctx:
        pers = ctx.enter_context(tc.tile_pool(name="pers", bufs=1))
        aith = [pers.tile([P, B], F32R, name=f"aith{k}") for k in range(KT)]
        h8a = pers.tile([P, KT, B], F8, name="h8a")
        l8a = pers.tile([P, KT, B], F8, name="l8a")
        gm = pers.tile([P, B], F32, name="gm")
        mxbc = pers.tile([P, B], F32, name="mxbc")
        ident = pers.tile([P, P], F32, name="ident")
        ones = pers.tile([P, P], BF16, name="ones")
        den = pers.tile([1, B], F32, name="den")
        make_identity(nc, ident[:])
        nc.gpsimd.memset(gm[:], -1e30)
        nc.gpsimd.memset(ones[:], 1.0)
        nc.gpsimd.memset(den[:], 0.0)

        # ---- phase 0: A^T = M^T X_i^T (hh f32r + one fp8 DR cross sweep) ----
        with ExitStack() as p0:
            wp = p0.enter_context(tc.tile_pool(name="wp", bufs=2))
            xsp = p0.enter_context(tc.tile_pool(name="xsp", bufs=3))
            xip = p0.enter_context(tc.tile_pool(name="xip", bufs=1))
            atp = p0.enter_context(tc.tile_pool(name="atp", bufs=2))
            ps0 = p0.enter_context(tc.tile_pool(name="ps0", bufs=6, space="PSUM"))

            # d=0 weight DMA + cast first on their queues: the first matmul
            # must not wait behind the X streams
            wf0 = wp.tile([P, KT, P], F32, name="wf0", tag="wf")
            nc.sync.dma_start(wf0[:], mh.ap()[0])
            hw0 = wp.tile([P, KT, P], F32R, name="hw0", tag="hw")
            nc.scalar.copy(hw0[:], wf0[:])
            m8t0 = wp.tile([P, KT, 2, P], F8, name="m8t0", tag="m8t")
            nc.sync.dma_start(m8t0[:], m8.ap()[0])

            # X_i^T moving operands, streamed HALF-FIRST so the d-loop's
            # half-0 matmuls start after only B/2 columns of X have landed
            hxi = xip.tile([P, KT, B], F32R, name="hxi")
            x8t = xip.tile([P, KT, 2, B], F8, name="x8t")
            for (lo, hi) in _chunks(B):
                for g in range(KT):
                    stg = xsp.tile([P, 512], F32, name=f"xstg{g}_{lo}", tag="xstg")
                    nc.sync.dma_start(stg[:], xih.ap()[g, :, lo:hi])
                    nc.scalar.copy(hxi[:, g, lo:hi], stg[:])
                    nc.sync.dma_start(x8t[:, g, :, lo:hi], x8p.ap()[g, :, :, lo:hi])

            for d in range(KT):
                if d == 0:
                    hw, m8t = hw0, m8t0
                else:
                    wf = wp.tile([P, KT, P], F32, name=f"wf{d}", tag="wf")
                    nc.sync.dma_start(wf[:], mh.ap()[d])
                    hw = wp.tile([P, KT, P], F32R, name=f"hw{d}", tag="hw")
                    nc.scalar.copy(hw[:], wf[:])
                    m8t = wp.tile([P, KT, 2, P], F8, name=f"m8t{d}", tag="m8t")
                    nc.sync.dma_start(m8t[:], m8.ap()[d])
                for (lo, hi) in _chunks(B):
                    ph = ps0.tile([P, 512], F32, name=f"ph{d}_{lo}", tag="pm")
                    pc = ps0.tile([P, 512], F32, name=f"pc{d}_{lo}", tag="pm")
                    for r in range(KT):
                        nc.tensor.matmul(ph[:], hw[:, r, :], hxi[:, r, lo:hi],
                                         start=(r == 0), stop=(r == KT - 1))
                    for r in range(KT):
                        nc.tensor.matmul(pc[:], m8t[:, r, :, :],
                                         x8t[:, r, :, lo:hi],
                                         start=(r == 0), stop=(r == KT - 1),
                                         perf_mode=DR)
                    at = atp.tile([P, 512], F32, name=f"at{d}_{lo}", tag="at")
                    # two PSUM reads aren't allowed in one instruction:
                    # ScalarE drains the cross PSUM (with the descale),
                    # DVE adds the hh PSUM
                    nc.scalar.mul(at[:], pc[:], CROSS_DESCALE)
                    nc.vector.tensor_add(at[:], at[:], ph[:])
                    nc.vector.tensor_copy(aith[d][:, lo:hi], at[:])
                    nc.scalar.mul(h8a[:, d, lo:hi],
                                  aith[d][:, lo:hi].bitcast(F32), S_HA)
                    nc.vector.tensor_sub(at[:], at[:],
                                         aith[d][:, lo:hi].bitcast(F32))
                    nc.vector.tensor_scalar_mul(l8a[:, d, lo:hi], at[:], S_LA)

        # ---- flash over key super-blocks (PE-local stats; see docstring) ----
        accp = ctx.enter_context(tc.tile_pool(name="accp", bufs=1))
        acc = [accp.tile([P, D], F32, name=f"acc{t}") for t in range(MT)]
        for t in range(MT):
            nc.gpsimd.memset(acc[t][:], 0.0)
        sp = ctx.enter_context(tc.tile_pool(name="sp", bufs=2 * sbn + 2))
        erp = ctx.enter_context(tc.tile_pool(name="erp", bufs=2 * sbn))
        xap = ctx.enter_context(tc.tile_pool(name="xap", bufs=3))
        xarp = ctx.enter_context(tc.tile_pool(name="xarp", bufs=2 * sbn))
        xthp = ctx.enter_context(tc.tile_pool(name="xthp", bufs=3))
        stat = ctx.enter_context(tc.tile_pool(name="stat", bufs=2))
        ps_s = ctx.enter_context(tc.tile_pool(name="ps_s", bufs=2, space="PSUM"))
        ps_o = ctx.enter_context(tc.tile_pool(name="ps_o", bufs=2, space="PSUM"))
        ps_d = ctx.enter_context(tc.tile_pool(name="ps_d", bufs=1, space="PSUM"))
        ps_t = ps_s

        def prep_block(s):
            xsplit = []
            for j in range(s * sbn, (s + 1) * sbn):
                xt_t = xap.tile([P, D], F32, name=f"xt{j}", tag="stg")
                nc.sync.dma_start(xt_t[:], xtj.ap()[j])
                xth = xthp.tile([P, D], F32R, name=f"xth{j}", tag="xth")
                nc.scalar.copy(xth[:], xt_t[:])
                l8x = xthp.tile([P, D], F8, name=f"l8x{j}", tag="l8x")
                nc.sync.dma_start(l8x[:], xl8.ap()[j])
                h8x = xthp.tile([P, D], F8, name=f"h8x{j}", tag="h8x")
                nc.sync.dma_start(h8x[:], xh8.ap()[j])
                xsplit.append((xth, l8x, h8x))
            return xsplit

        def s_burst(s, xsplit):
            ssb = []
            for idx, j in enumerate(range(s * sbn, (s + 1) * sbn)):
                xth, l8x, h8x = xsplit[idx]
                s_t = sp.tile([P, B], F32, name=f"s{j}", tag="s")
                pss = [ps_s.tile([P, 512], F32, name=f"pss{j}_{c}", tag="pss")
                       for c in range(2)]
                # f32r hh pass, both chunks back-to-back (same PE mode)
                for c, (lo, hi) in enumerate(_chunks(B)):
                    for k in range(KT):
                        nc.tensor.matmul(pss[c][:], xth[:, k * P:(k + 1) * P],
                                         aith[k][:, lo:hi], start=(k == 0), stop=(k == KT - 1))
                # fp8 DoubleRow cross passes: accumulate onto the closed f32r
                # group via has_written (start=False); stop is sim-only
                # bookkeeping so every DR matmul closes itself. Chunk 0
                # drains (copy + running max) while chunk 1's fp8 matmuls
                # stream, so the stats transposes can start the moment the
                # burst ends.
                for c, (lo, hi) in enumerate(_chunks(B)):
                    for u in range(NPAIR):
                        nc.tensor.matmul(pss[c][:], pair_st(l8x, u),
                                         h8a[:, 2 * u:2 * u + 2, lo:hi],
                                         start=False, stop=True, perf_mode=DR,
                                         skip_group_check=True)
                    for u in range(NPAIR):
                        nc.tensor.matmul(pss[c][:], pair_st(h8x, u),
                                         l8a[:, 2 * u:2 * u + 2, lo:hi],
                                         start=False, stop=True, perf_mode=DR,
                                         skip_group_check=True)
                    nc.scalar.copy(s_t[:, lo:hi], pss[c][:])
                    nc.vector.tensor_max(gm[:, lo:hi], gm[:, lo:hi], pss[c][:])
                ssb.append(s_t)
            return ssb

        def stats_block(s, omx):
            # per-query-column running max (transpose-reduce gm chunks)
            nmx = stat.tile([P, MT], F32, name=f"nmx{s}", tag="nmx")
            corr = stat.tile([P, MT], F32, name=f"corr{s}", tag="corr")
            for c in range(MT):
                pt = ps_t.tile([P, P], F32, name=f"pt{s}_{c}", tag="pss")
                nc.tensor.transpose(pt[:], gm[:, c * P:(c + 1) * P], ident[:])
                nc.vector.reduce_max(nmx[:, c:c + 1], pt[:], axis=AXX)
            if omx is None:
                nc.vector.memset(corr[:], 0.0)
            else:
                dmx = stat.tile([P, MT], F32, name=f"dmx{s}", tag="dmx")
                nc.vector.tensor_sub(dmx[:], omx[:], nmx[:])
                nc.scalar.activation(corr[:], dmx[:], EXP)

            # broadcast nmx (query-major) -> mxbc [P, B] (key-major free)
            ptb = ps_t.tile([P, P], F32, name=f"ptb{s}", tag="pss")
            nc.tensor.transpose(ptb[:MT, :], nmx[:], ident[:])
            mtmp = stat.tile([MT, P], F32, name=f"mtmp{s}", tag="mtmp")
            nc.scalar.copy(mtmp[:], ptb[:MT, :])
            # issue the tiny mrow DMA from the scalar queue so it doesn't
            # sit behind the bulk xtj/xa loads on the sync queue
            mrow = stat.tile([1, B], F32, name=f"mrow{s}", tag="mrow", bufs=1)
            nc.scalar.dma_start(mrow[:].rearrange("a (b c) -> a b c", b=MT), mtmp[:])
            nc.gpsimd.partition_broadcast(mxbc[:], mrow[:])
            return nmx, corr

        def exp_block(s, ssb):
            # E = exp(S - max), exp writes bf16 er (out dtype converts).
            # Chunked [P, 512] with chunk 0 of every tile first: the out
            # burst consumes er columns t*128.. in t order, so all its
            # early stationaries come from chunk 0 — this halves the time
            # from max-broadcast to out-burst start.
            ers = [erp.tile([P, B], BF16, name=f"er{s}_{idx}", tag="er")
                   for idx in range(len(ssb))]
            for (lo, hi) in _chunks(B):
                for idx, s_t in enumerate(ssb):
                    nc.vector.tensor_sub(s_t[:, lo:hi], s_t[:, lo:hi], mxbc[:, lo:hi])
                    nc.scalar.activation(ers[idx][:, lo:hi], s_t[:, lo:hi], EXP)
            return ers

        def xar_block(s):
            xar = []
            for j in range(s * sbn, (s + 1) * sbn):
                xa_t = xarp.tile([P, D], BF16, name=f"xa{j}", tag="xar")
                nc.sync.dma_start(xa_t[:], xa.ap()[j * P:(j + 1) * P, :])
                xar.append(xa_t)
            return xar

        def den_update(s, ers, corr):
            # softmax denominator via ones-stationary matmuls (row layout),
            # rescaled with crow = corr transposed to row-major (the same
            # transpose/flatten-DMA idiom as the mrow broadcast)
            dps = ps_d.tile([P, B], F32, name=f"dps{s}", tag="dps")
            for (lo, hi) in _chunks(B):
                for idx in range(sbn):
                    nc.tensor.matmul(dps[:, lo:hi], ones[:], ers[idx][:, lo:hi],
                                     start=(idx == 0), stop=(idx == sbn - 1))
            ptc = ps_t.tile([P, P], F32, name=f"ptc{s}", tag="pss")
            nc.tensor.transpose(ptc[:MT, :], corr[:], ident[:])
            ctmp = stat.tile([MT, P], F32, name=f"ctmp{s}", tag="mtmp")
            nc.scalar.copy(ctmp[:], ptc[:MT, :])
            crow = stat.tile([1, B], F32, name=f"crow{s}", tag="crow", bufs=1)
            nc.scalar.dma_start(crow[:].rearrange("a (b c) -> a b c", b=MT), ctmp[:])
            nc.vector.tensor_mul(den[:], den[:], crow[:])
            nc.vector.tensor_add(den[:], den[:], dps[0:1, :])

        def out_block(s, ers, xar, corr, final=False):
            # out accumulation: acc = acc*corr + E^T @ X (bf16 burst).
            # On the final block the denominator runs FIRST so its
            # row->query-major DRAM bounce finishes during the burst and the
            # per-tile divides+stores pipeline with the matmuls.
            rcd = None
            if final:
                den_update(s, ers, corr)
                nc.sync.dma_start(dscr.ap()[:, :], den[:])
                dtmp = stat.tile([P, P], F32, name="dtmp", tag="dtmp")
                nc.gpsimd.memset(dtmp[:], 0.0)
                nc.sync.dma_start(dtmp[:MT, :], dscr.ap()[0, :].rearrange("(b c) -> b c", b=MT))
            for t in range(MT):
                po = ps_o.tile([P, D], F32, name=f"po{s}_{t}", tag="po")
                # idx outer so the column chunks reuse one stationary
                # operand back-to-back (LDWEIGHTS locality); each chunk's
                # PSUM accumulation group still spans idx 0..sbn-1
                for idx in range(sbn):
                    er = ers[idx][:]
                    for (lo, hi) in _chunks(D):
                        nc.tensor.matmul(po[:, lo:hi], er[:, t * P:(t + 1) * P],
                                         xar[idx][:, lo:hi], start=(idx == 0), stop=(idx == sbn - 1))
                nc.vector.scalar_tensor_tensor(acc[t][:], acc[t][:],
                                               corr[:, t:t + 1], po[:],
                                               op0=ALU.mult, op1=ALU.add)
                if final and t == 2:
                    ptd = ps_t.tile([P, P], F32, name="ptd", tag="pss")
                    nc.tensor.transpose(ptd[:], dtmp[:], ident[:])
                    rcd = stat.tile([P, MT], F32, name="rcd", tag="rcd")
                    nc.vector.reciprocal(rcd[:], ptd[:, :MT])
                if final and t >= 2:
                    for tt in ([0, 1, 2] if t == 2 else [t]):
                        nc.vector.tensor_scalar_mul(acc[tt][:], acc[tt][:], rcd[:, tt:tt + 1])
                        eng = nc.sync if tt % 2 == 0 else nc.scalar
                        eng.dma_start(out.ap()[tt * P:(tt + 1) * P, :], acc[tt][:])
            if not final:
                den_update(s, ers, corr)

        omx = None
        prev = None     # out_block args for block s-1
        xsplit = prep_block(0)
        for s in range(NSB):
            ssb = s_burst(s, xsplit)
            if s + 1 < NSB:
                xsplit = prep_block(s + 1)
            # stats(s) on PE right after the burst (its gm maxes already
            # drained), so the max-broadcast/exp chain overlaps out(s-1)
            nmx, corr = stats_block(s, omx)
            omx = nmx
            if prev is not None:
                out_block(s - 1, *prev)
            ers = exp_block(s, ssb)
            xar = xar_block(s)
            prev = (ers, xar, corr)
        out_block(NSB - 1, *prev, final=True)

    nc.compile()
    return nc


def _split_f32r(x):
    """Host replica of the f32r hi/lo split: hi = x rounded (half-up) to an
    11-bit mantissa — so the PE's e10m11 operand truncation and the
    on-device f32r casts both read it back exactly — and lo = x - hi,
    exact in fp32."""
    x = np.ascontiguousarray(x, np.float32)
    hi = ((x.view(np.uint32) + np.uint32(0x800)) & np.uint32(0xFFFFF000)).view(np.float32)
    return hi, (x - hi).astype(np.float32)


def _to_f8(x):
    import ml_dtypes
    return np.clip(x, -240.0, 240.0).astype(ml_dtypes.float8_e4m3)


def prep_inputs(X, Wq, Wk, S, D, n_cores):
    import ml_dtypes
    B = S // n_cores
    NT = S // P
    KT = D // P
    X = np.ascontiguousarray(X, np.float32)
    scale = 1.0 / np.sqrt(D)

    # flash operands (keys side): X transposed tiles, 11-bit hi + fp8 splits
    xtj = np.ascontiguousarray(
        X.reshape(NT, P, KT, P).transpose(0, 3, 2, 1).reshape(NT, P, D))
    xtj_hi, xtj_lo = _split_f32r(xtj)
    xl8 = _to_f8(xtj_lo * np.float32(S_LX))
    xh8 = _to_f8(xtj_hi * np.float32(S_HX))
    xa = X.astype(ml_dtypes.bfloat16)

    # host weight fusion: M = (Wq/sqrt(D)) @ Wk^T in fp64, then 11-bit hi
    M = np.asarray(
        (np.asarray(Wq, np.float64) * scale) @ np.asarray(Wk, np.float64).T,
        np.float32)
    mhi, mlo = _split_f32r(M)
    # device layouts: mh_dev[d, p, g, c] = Mh[g*P+p, d*P+c] (contiguous per d)
    def to_dpgc(W):
        return np.ascontiguousarray(
            W.reshape(KT, P, KT, P).transpose(2, 1, 0, 3))
    mh_dev = to_dpgc(mhi)
    m8_dev = np.ascontiguousarray(np.stack(
        [to_dpgc(_to_f8(mlo * np.float32(S_ML))),
         to_dpgc(_to_f8(mhi * np.float32(S_MH8)))], axis=3))

    # phase-0 moving operands: X^T core slices, 11-bit hi + fp8 pair
    XT = np.ascontiguousarray(X.T)
    xih_full, xil_full = _split_f32r(XT)
    xh8p_full = _to_f8(xih_full * np.float32(S_XH8))
    xl8p_full = _to_f8(xil_full * np.float32(S_XL8))

    in_maps = []
    for i in range(n_cores):
        sl = slice(i * B, (i + 1) * B)
        in_maps.append({
            "xtj": xtj_hi, "xl8": xl8, "xh8": xh8, "xa": xa,
            "mh": mh_dev, "m8": m8_dev,
            "xih": np.ascontiguousarray(xih_full[:, sl]).reshape(KT, P, B),
            "x8p": np.ascontiguousarray(np.stack(
                [xh8p_full[:, sl], xl8p_full[:, sl]],
                axis=1).reshape(KT, P, 2, B)),
        })
    return in_maps


_CACHE = {}


def _get_kernel(S, D, B, sbn):
    key = (S, D, B, sbn)
    if key not in _CACHE:
        _CACHE[key] = build_core_kernel(S, D, B, sbn=sbn)
    return _CACHE[key]


def kernel(inputs, weight_query, weight_key):
    S, D = inputs.shape
    assert (S, D) == (SEQ, DIM)
    B = S // NCORES
    nc = _get_kernel(S, D, B, SBN)
    in_maps = prep_inputs(inputs, weight_query, weight_key, S, D, NCORES)
    res = run_bass_kernel_spmd(nc, in_maps, core_ids=list(range(NCORES)))
    return np.concatenate([res.results[i]["out"] for i in range(NCORES)], axis=0)


if __name__ == "__main__":
    rng = np.random.default_rng(0)
    X = rng.standard_normal((SEQ, DIM), dtype=np.float32)
    Wq = rng.standard_normal((DIM, DIM), dtype=np.float32)
    Wk = rng.standard_normal((DIM, DIM), dtype=np.float32)
    out = kernel(X, Wq, Wk)
    print(out.shape, out.dtype)
